# revision 6
# baseline (speedup 1.0000x reference)
"""AnchorDML Trainium2 kernel: 8-core SPMD, data-parallel over x rows with
sharded anchor encoding + AllGather of encoded anchors.

Problem (hardcoded):
    N, M, D, C = 8192, 4096, 512, 100
    xe = mish(mish(x @ W1 + b1) @ W2 + b2)          [N, D]
    se = mish(mish(samples @ W1 + b1) @ W2 + b2)    [M, D]
    dist = sqrt(max(|xe|^2 + |se|^2 - 2 xe@se.T, 0))  [N, M]
    out = log_softmax(tanh(dist @ Wp + bp), axis=1)   [N, C]

Sharding: core g handles x rows [1024g, 1024(g+1)) and encodes anchors
[512g, 512(g+1)); encoded (scaled) anchors + |se|^2 are AllGathered.

mish(v) = v * tanh(softplus(v)) is computed from exp/square primitives
(no mish LUT in this compiler build):
    u = e^v;  tanh(softplus(v)) = 1 - 2/((u+1)^2+1)
    mish(v) = v - 2*v / ((u+1)^2 + 1)
Biases ride the matmuls as rank-1 augmentations so psum already holds
the pre-activation v.
"""
import numpy as np
from concourse import bass, bacc, tile, mybir, bass_utils, masks

N, M, D, C = 8192, 4096, 512, 100
NCORES = 8
RPC = N // NCORES      # 1024 x-rows per core
MPC = M // NCORES      # 512 anchors encoded per core
KD = D // 128          # 4 contraction chunks of 128
AGROWS = D + 1         # 512 rows of -2*se^T plus one row of s2
NMT = M // 128         # 32 anchor tiles in the distance matmul
NRC = RPC // 512       # 2 row-chunks of 512

F32 = mybir.dt.float32
F32R = mybir.dt.float32r
AF = mybir.ActivationFunctionType
ALU = mybir.AluOpType


def build_kernel():
    nc = bacc.Bacc("TRN2", target_bir_lowering=False, debug=False,
                   num_devices=NCORES)

    xT = nc.dram_tensor("xT", [D, RPC], F32, kind="ExternalInput")
    sT = nc.dram_tensor("sT", [D, MPC], F32, kind="ExternalInput")
    W1 = nc.dram_tensor("W1", [D, D], F32, kind="ExternalInput")
    W2 = nc.dram_tensor("W2", [D, D], F32, kind="ExternalInput")
    b1 = nc.dram_tensor("b1", [1, D], F32, kind="ExternalInput")
    b2 = nc.dram_tensor("b2", [1, D], F32, kind="ExternalInput")
    Wp = nc.dram_tensor("Wp", [M, C], F32, kind="ExternalInput")
    bp = nc.dram_tensor("bp", [1, C], F32, kind="ExternalInput")
    out = nc.dram_tensor("out", [RPC, C], F32, kind="ExternalOutput")

    with tile.TileContext(nc) as tc:
        _body(tc, xT, sT, W1, W2, b1, b2, Wp, bp, out)

    nc.compile()
    return nc


def _body(tc, xT, sT, W1, W2, b1, b2, Wp, bp, out):
    nc = tc.nc
    with (
        tc.tile_pool(name="const", bufs=1) as const,
        tc.tile_pool(name="wpool", bufs=1) as wpool,
        tc.tile_pool(name="spool", bufs=1) as spool,
        tc.tile_pool(name="xpool", bufs=1) as xpool,
        tc.tile_pool(name="gpool", bufs=1) as gpool,
        tc.tile_pool(name="mpool", bufs=2) as mpool,
        tc.tile_pool(name="dpool", bufs=3) as dpool,
        tc.tile_pool(name="zpool", bufs=2) as zpool,
        tc.tile_pool(name="ps", bufs=1, space="PSUM") as ps,
        tc.tile_pool(name="psz", bufs=2, space="PSUM") as psz,
        tc.tile_pool(name="dram", bufs=1, space="DRAM") as dram,
    ):
        # ---- constants ----
        ident = const.tile([C, C], F32)
        masks.make_identity(nc, ident[:])
        ones_f32 = const.tile([128, 1], F32)
        nc.gpsimd.memset(ones_f32[:], 1.0)
        ones_col = const.tile([128, 1], F32R)     # lhsT for row-sum matmuls
        nc.scalar.activation(ones_col[:], ones_f32[:], AF.Copy)
        onesr_f32 = const.tile([1, 512], F32)
        nc.gpsimd.memset(onesr_f32[:], 1.0)
        ones512 = const.tile([1, 512], F32R)      # rhs for bias augmentation
        nc.scalar.activation(ones512[:], onesr_f32[:], AF.Copy)

        # ---- weight loads (f32 -> f32r casting DMAs on gpsimd) ----
        W1_sb = wpool.tile([128, KD, D], F32R)
        W2_sb = wpool.tile([128, KD, D], F32R)
        for k in range(KD):
            nc.gpsimd.dma_start(W1_sb[:, k, :], W1[128 * k:128 * (k + 1), :])
            nc.gpsimd.dma_start(W2_sb[:, k, :], W2[128 * k:128 * (k + 1), :])
        b1_sb = wpool.tile([1, D], F32R)
        b2_sb = wpool.tile([1, D], F32R)
        nc.gpsimd.dma_start(b1_sb[:], b1[:])
        nc.gpsimd.dma_start(b2_sb[:], b2[:])
        bp_sb = wpool.tile([1, C], F32R)
        nc.gpsimd.dma_start(bp_sb[:], bp[:])
        Wp_sb = wpool.tile([128, NMT, C], F32R)
        for t in range(NMT):
            nc.gpsimd.dma_start(Wp_sb[:, t, :], Wp[128 * t:128 * (t + 1), :])

        sT_sb = spool.tile([128, KD, MPC], F32R)
        for k in range(KD):
            nc.gpsimd.dma_start(sT_sb[:, k, :], sT[128 * k:128 * (k + 1), :])
        xT_sb = xpool.tile([128, KD, RPC], F32R)
        for k in range(KD):
            nc.gpsimd.dma_start(xT_sb[:, k, :], xT[128 * k:128 * (k + 1), :])

        def enc_layer(dst, dst_slice, Wsb, brow, src, src_slice, f, width):
            """dst[:, f, dst_slice] = mish(src.T @ W + b) for one 128-wide
            feature chunk; contraction over src rows (+ bias row)."""
            vps = ps.tile([128, 512], F32, tag="mm", bufs=3)
            vp = vps[:, :width]
            for k in range(KD):
                nc.tensor.matmul(vp, Wsb[:, k, 128 * f:128 * (f + 1)],
                                 src[:, k, src_slice],
                                 start=(k == 0), stop=False)
            nc.tensor.matmul(vp, brow[:, 128 * f:128 * (f + 1)],
                             ones512[:, :width], start=False, stop=True)
            # mish from primitives
            u = mpool.tile([128, 512], F32, tag="mu")
            nc.scalar.activation(u[:, :width], vp, AF.Exp)
            w = mpool.tile([128, 512], F32, tag="mw")
            nc.scalar.activation(w[:, :width], u[:, :width], AF.Square,
                                 bias=1.0)
            w1 = mpool.tile([128, 512], F32, tag="mw1")
            nc.vector.tensor_scalar_add(w1[:, :width], w[:, :width], 1.0)
            r = mpool.tile([128, 512], F32, tag="mr")
            nc.vector.reciprocal(r[:, :width], w1[:, :width])
            q = mpool.tile([128, 512], F32, tag="mq")
            nc.vector.scalar_tensor_tensor(q[:, :width], r[:, :width], -2.0,
                                           vp, op0=ALU.mult, op1=ALU.mult)
            nc.vector.tensor_tensor(dst[:, f, dst_slice], vp, q[:, :width],
                                    op=ALU.add)

        # ---- encode local anchors, kept transposed [feat, m] ----
        hs_sb = spool.tile([128, KD, MPC], F32R)
        for f in range(KD):
            enc_layer(hs_sb, slice(0, MPC), W1_sb, b1_sb, sT_sb,
                      slice(0, MPC), f, MPC)
        se_sb = spool.tile([128, KD, MPC], F32R)
        for f in range(KD):
            enc_layer(se_sb, slice(0, MPC), W2_sb, b2_sb, hs_sb,
                      slice(0, MPC), f, MPC)

        # s2 row: [1, MPC] = sum_d se^2, via ones-vector matmul
        sqse_sb = spool.tile([128, KD, MPC], F32R, tag="hs_sb")
        nc.scalar.activation(sqse_sb[:, :, :], se_sb[:, :, :], AF.Square)
        s2ps = ps.tile([1, 512], F32, tag="row", bufs=1)
        for k in range(KD):
            nc.tensor.matmul(s2ps[:], ones_col[:], sqse_sb[:, k, :],
                             start=(k == 0), stop=(k == KD - 1))
        s2row_sb = spool.tile([1, MPC], F32)
        nc.scalar.activation(s2row_sb[:], s2ps[:], AF.Copy)

        # seA = -2 * se (the gathered distance-matmul stationary operand)
        seA_sb = spool.tile([128, KD, MPC], F32R, tag="sT_sb")
        nc.vector.tensor_scalar_mul(seA_sb[:, :, :], se_sb[:, :, :], -2.0)

        # ---- AllGather of [seA ; s2] ----
        ag_in = dram.tile([AGROWS, MPC], F32)
        ag_out = dram.tile([NCORES * AGROWS, MPC], F32, addr_space="Shared")
        for k in range(KD):
            nc.gpsimd.dma_start(ag_in[128 * k:128 * (k + 1), :],
                                seA_sb[:, k, :])
        nc.sync.dma_start(ag_in[D:D + 1, :], s2row_sb[:])
        nc.gpsimd.collective_compute(
            "AllGather",
            mybir.AluOpType.bypass,
            replica_groups=[list(range(NCORES))],
            ins=[ag_in.opt()],
            outs=[ag_out.opt()],
        )

        # ---- encode x rows (overlaps the AllGather) ----
        hx_sb = xpool.tile([128, KD, RPC], F32R)
        for f in range(KD):
            for rc in range(NRC):
                enc_layer(hx_sb, slice(512 * rc, 512 * (rc + 1)), W1_sb,
                          b1_sb, xT_sb, slice(512 * rc, 512 * (rc + 1)),
                          f, 512)
        xe_sb = xpool.tile([128, KD, RPC], F32R, tag="xT_sb")
        for f in range(KD):
            for rc in range(NRC):
                enc_layer(xe_sb, slice(512 * rc, 512 * (rc + 1)), W2_sb,
                          b2_sb, hx_sb, slice(512 * rc, 512 * (rc + 1)),
                          f, 512)

        # x2 row: [1, RPC] = sum_d xe^2
        sqxe_sb = xpool.tile([128, KD, RPC], F32R, tag="hx_sb")
        nc.scalar.activation(sqxe_sb[:, :, :], xe_sb[:, :, :], AF.Square)
        x2row_sb = xpool.tile([1, RPC], F32R)
        for rc in range(NRC):
            xps = ps.tile([1, 512], F32, tag="row", bufs=1)
            for k in range(KD):
                nc.tensor.matmul(xps[:], ones_col[:],
                                 sqxe_sb[:, k, 512 * rc:512 * (rc + 1)],
                                 start=(k == 0), stop=(k == KD - 1))
            nc.scalar.activation(x2row_sb[:, 512 * rc:512 * (rc + 1)],
                                 xps[:], AF.Copy)

        # ---- load gathered anchors: seAg[d_part, k, m] for all 4096 m ----
        seAg_sb = gpool.tile([128, KD, M], F32R)
        for k in range(KD):
            for g in range(NCORES):
                nc.gpsimd.dma_start(
                    seAg_sb[:, k, MPC * g:MPC * (g + 1)],
                    ag_out[AGROWS * g + 128 * k:AGROWS * g + 128 * (k + 1), :])
        # s2 columns: s2c[p, t] = s2[128 t + p]
        s2c_sb = gpool.tile([128, NMT], F32)
        for g in range(NCORES):
            row = ag_out[AGROWS * g + D:AGROWS * g + D + 1, :]
            nc.sync.dma_start(
                s2c_sb[:, (MPC // 128) * g:(MPC // 128) * (g + 1)],
                row.rearrange("o (f p) -> (o p) f", p=128))

        # ---- main fused loop: distances + perceptron (transposed) ----
        zpre_sb = zpool.tile([128, 2 * NRC * 2, C], F32, bufs=1)
        for rc in range(NRC):
            zt_ps = psz.tile([C, 512], F32)
            for t in range(NMT):
                d2ps = ps.tile([128, 512], F32, tag="mm", bufs=3)
                for k in range(KD):
                    nc.tensor.matmul(d2ps[:],
                                     seAg_sb[:, k, 128 * t:128 * (t + 1)],
                                     xe_sb[:, k, 512 * rc:512 * (rc + 1)],
                                     start=(k == 0), stop=False)
                # += x2[r] (rank-1 augmentation)
                nc.tensor.matmul(d2ps[:], ones512[:, :128],
                                 x2row_sb[:, 512 * rc:512 * (rc + 1)],
                                 start=False, stop=True)
                distT = dpool.tile([128, 512], F32R)
                nc.scalar.activation(distT[:], d2ps[:], AF.Sqrt,
                                     bias=s2c_sb[:, t:t + 1])
                nc.tensor.matmul(zt_ps[:], Wp_sb[:, t, :], distT[:],
                                 start=(t == 0), stop=False,
                                 skip_group_check=True)
            # += bp (rank-1 augmentation), closes the accumulation
            nc.tensor.matmul(zt_ps[:], bp_sb[:], ones512[:],
                             start=False, stop=True, skip_group_check=True)
            # transpose pre-tanh z to [r, C] blocks
            zt_sb = zpool.tile([C, 512], F32, bufs=1)
            nc.vector.tensor_copy(zt_sb[:], zt_ps[:])
            for j in range(4):
                ztr = ps.tile([128, C], F32, tag="tr", bufs=2)
                nc.tensor.matmul(ztr[:], zt_sb[:, 128 * j:128 * (j + 1)],
                                 ident[:], is_transpose=True)
                nc.vector.tensor_copy(zpre_sb[:, 4 * rc + j, :], ztr[:])

        # ---- epilogue: tanh + log-softmax (batched to minimize ACT table
        # switches) ----
        NT = 2 * NRC * 2  # 8 tiles of 128 rows
        zth_sb = zpool.tile([128, NT, C], F32, bufs=1)
        nc.scalar.activation(zth_sb[:, :, :], zpre_sb[:, :, :], AF.Tanh)
        e_sb = zpool.tile([128, NT, C], F32, bufs=1)
        nc.scalar.activation(e_sb[:, :, :], zth_sb[:, :, :], AF.Exp)
        ssum = zpool.tile([128, NT], F32, bufs=1)
        nc.vector.tensor_reduce(ssum[:], e_sb[:, :, :],
                                axis=mybir.AxisListType.X, op=ALU.add)
        lns = zpool.tile([128, NT], F32, bufs=1)
        nc.scalar.activation(lns[:], ssum[:], AF.Ln)
        for jj in range(NT):
            o_sb = zpool.tile([128, C], F32)
            nc.vector.tensor_scalar(o_sb[:], zth_sb[:, jj, :],
                                    lns[:, jj:jj + 1], None,
                                    op0=ALU.subtract)
            nc.sync.dma_start(out[128 * jj:128 * (jj + 1), :], o_sb[:])


_NC_CACHE = None


def _get_nc():
    global _NC_CACHE
    if _NC_CACHE is None:
        _NC_CACHE = build_kernel()
    return _NC_CACHE


def make_in_maps(x, samples, W1, b1, W2, b2, Wp, bp):
    xT = np.ascontiguousarray(np.asarray(x, dtype=np.float32).T)
    sT = np.ascontiguousarray(np.asarray(samples, dtype=np.float32).T)
    W1 = np.ascontiguousarray(np.asarray(W1, dtype=np.float32))
    W2 = np.ascontiguousarray(np.asarray(W2, dtype=np.float32))
    Wp = np.ascontiguousarray(np.asarray(Wp, dtype=np.float32))
    b1c = np.ascontiguousarray(np.asarray(b1, dtype=np.float32).reshape(1, D))
    b2c = np.ascontiguousarray(np.asarray(b2, dtype=np.float32).reshape(1, D))
    bpc = np.ascontiguousarray(np.asarray(bp, dtype=np.float32).reshape(1, C))
    in_maps = []
    for g in range(NCORES):
        in_maps.append({
            "xT": np.ascontiguousarray(xT[:, RPC * g:RPC * (g + 1)]),
            "sT": np.ascontiguousarray(sT[:, MPC * g:MPC * (g + 1)]),
            "W1": W1, "W2": W2, "b1": b1c, "b2": b2c,
            "Wp": Wp, "bp": bpc,
        })
    return in_maps


def run(in_maps, trace=False):
    nc = _get_nc()
    res = bass_utils.run_bass_kernel_spmd(nc, in_maps,
                                          core_ids=list(range(NCORES)),
                                          trace=trace)
    outp = np.concatenate([res.results[g]["out"] for g in range(NCORES)],
                          axis=0).astype(np.float32)
    return outp, res


def kernel(x, samples, W1, b1, W2, b2, Wp, bp):
    in_maps = make_in_maps(x, samples, W1, b1, W2, b2, Wp, bp)
    outp, _ = run(in_maps, trace=False)
    return outp


# revision 7
# speedup vs baseline: 1.0225x; 1.0225x over previous
"""AnchorDML Trainium2 kernel: 8-core SPMD, data-parallel over x rows with
sharded anchor encoding + AllGather of encoded anchors.

Problem (hardcoded):
    N, M, D, C = 8192, 4096, 512, 100
    xe = mish(mish(x @ W1 + b1) @ W2 + b2)          [N, D]
    se = mish(mish(samples @ W1 + b1) @ W2 + b2)    [M, D]
    dist = sqrt(max(|xe|^2 + |se|^2 - 2 xe@se.T, 0))  [N, M]
    out = log_softmax(tanh(dist @ Wp + bp), axis=1)   [N, C]

Sharding: core g handles x rows [1024g, 1024(g+1)) and encodes anchors
[512g, 512(g+1)); encoded (scaled) anchors + |se|^2 are AllGathered.

mish(v) = v * tanh(softplus(v)) with softplus from exp/ln (no mish LUT
in this compiler build): sp = ln(1 + e^v), t = tanh(sp), mish = v*t.
Tanh lives in a different ACT table than ln/exp, so tanh is batched per
encoder layer chunk-group to amortize ACT table loads. Biases ride the
matmuls as rank-1 augmentations so psum holds the pre-activation v; v is
staged to SBUF immediately so psum banks recycle fast.
"""
import numpy as np
from concourse import bass, bacc, tile, mybir, bass_utils, masks

N, M, D, C = 8192, 4096, 512, 100
NCORES = 8
RPC = N // NCORES      # 1024 x-rows per core
MPC = M // NCORES      # 512 anchors encoded per core
KD = D // 128          # 4 contraction chunks of 128
AGROWS = D + 1         # 512 rows of -2*se^T plus one row of s2
NMT = M // 128         # 32 anchor tiles in the distance matmul
NRC = RPC // 512       # 2 row-chunks of 512

F32 = mybir.dt.float32
F32R = mybir.dt.float32r
AF = mybir.ActivationFunctionType
ALU = mybir.AluOpType


def build_kernel():
    nc = bacc.Bacc("TRN2", target_bir_lowering=False, debug=False,
                   num_devices=NCORES)

    xT = nc.dram_tensor("xT", [D, RPC], F32, kind="ExternalInput")
    sT = nc.dram_tensor("sT", [D, MPC], F32, kind="ExternalInput")
    W1 = nc.dram_tensor("W1", [D, D], F32, kind="ExternalInput")
    W2 = nc.dram_tensor("W2", [D, D], F32, kind="ExternalInput")
    b1 = nc.dram_tensor("b1", [1, D], F32, kind="ExternalInput")
    b2 = nc.dram_tensor("b2", [1, D], F32, kind="ExternalInput")
    Wp = nc.dram_tensor("Wp", [M, C], F32, kind="ExternalInput")
    bp = nc.dram_tensor("bp", [1, C], F32, kind="ExternalInput")
    out = nc.dram_tensor("out", [RPC, C], F32, kind="ExternalOutput")

    with tile.TileContext(nc) as tc:
        _body(tc, xT, sT, W1, W2, b1, b2, Wp, bp, out)

    nc.compile()
    return nc


def _body(tc, xT, sT, W1, W2, b1, b2, Wp, bp, out):
    nc = tc.nc
    with (
        tc.tile_pool(name="const", bufs=1) as const,
        tc.tile_pool(name="wpool", bufs=1) as wpool,
        tc.tile_pool(name="spool", bufs=1) as spool,
        tc.tile_pool(name="xpool", bufs=1) as xpool,
        tc.tile_pool(name="gpool", bufs=1) as gpool,
        tc.tile_pool(name="mpool", bufs=2) as mpool,
        tc.tile_pool(name="dpool", bufs=4) as dpool,
        tc.tile_pool(name="zpool", bufs=2) as zpool,
        tc.tile_pool(name="ps", bufs=1, space="PSUM") as ps,
        tc.tile_pool(name="psz", bufs=2, space="PSUM") as psz,
        tc.tile_pool(name="dram", bufs=1, space="DRAM") as dram,
    ):
        # ---- constants ----
        ident = const.tile([C, C], F32)
        masks.make_identity(nc, ident[:])
        ones_f32 = const.tile([128, 1], F32)
        nc.gpsimd.memset(ones_f32[:], 1.0)
        ones_col = const.tile([128, 1], F32R)     # lhsT for row-sum matmuls
        nc.scalar.activation(ones_col[:], ones_f32[:], AF.Copy)
        onesr_f32 = const.tile([1, 512], F32)
        nc.gpsimd.memset(onesr_f32[:], 1.0)
        ones512 = const.tile([1, 512], F32R)      # rhs for bias augmentation
        nc.scalar.activation(ones512[:], onesr_f32[:], AF.Copy)

        # ---- input loads: plain DMAs into f32r tiles (bitcast, no cast) ----
        W1_sb = wpool.tile([128, KD, D], F32R)
        W2_sb = wpool.tile([128, KD, D], F32R)
        for k in range(KD):
            nc.sync.dma_start(W1_sb[:, k, :],
                              W1[128 * k:128 * (k + 1), :].bitcast(F32R))
            nc.sync.dma_start(W2_sb[:, k, :],
                              W2[128 * k:128 * (k + 1), :].bitcast(F32R))
        b1_sb = wpool.tile([1, D], F32R)
        b2_sb = wpool.tile([1, D], F32R)
        nc.sync.dma_start(b1_sb[:], b1[:].bitcast(F32R))
        nc.sync.dma_start(b2_sb[:], b2[:].bitcast(F32R))
        bp_sb = wpool.tile([1, C], F32R)
        nc.sync.dma_start(bp_sb[:], bp[:].bitcast(F32R))
        Wp_sb = wpool.tile([128, NMT, C], F32R)
        for t in range(NMT):
            nc.sync.dma_start(Wp_sb[:, t, :],
                              Wp[128 * t:128 * (t + 1), :].bitcast(F32R))

        sT_sb = spool.tile([128, KD, MPC], F32R)
        for k in range(KD):
            nc.sync.dma_start(sT_sb[:, k, :],
                              sT[128 * k:128 * (k + 1), :].bitcast(F32R))
        xT_sb = xpool.tile([128, KD, RPC], F32R)
        for k in range(KD):
            nc.sync.dma_start(xT_sb[:, k, :],
                              xT[128 * k:128 * (k + 1), :].bitcast(F32R))

        def enc_layer(dst, Wsb, brow, src, rc, width):
            """dst[:, :, rc*width:(rc+1)*width] = mish(src.T @ W + b) for all
            KD feature chunks of one layer / row-chunk. Pre-activation v is
            staged to SBUF so psum recycles fast; sp=ln(1+e^v) lands in dst;
            tanh+multiply are batched over the KD chunks."""
            dsl = slice(width * rc, width * (rc + 1))
            vstage = mpool.tile([128, KD, 512], F32, tag="vstage")
            for f in range(KD):
                vps = ps.tile([128, 512], F32, tag="mm", bufs=4)
                vp = vps[:, :width]
                for k in range(KD):
                    nc.tensor.matmul(vp, Wsb[:, k, 128 * f:128 * (f + 1)],
                                     src[:, k, dsl],
                                     start=(k == 0), stop=False)
                nc.tensor.matmul(vp, brow[:, 128 * f:128 * (f + 1)],
                                 ones512[:, :width], start=False, stop=True)
                u = mpool.tile([128, 512], F32, tag="mu", bufs=3)
                nc.scalar.activation(u[:, :width], vp, AF.Exp)
                nc.vector.tensor_copy(vstage[:, f, :width], vp)
                nc.scalar.activation(dst[:, f, dsl], u[:, :width], AF.Ln,
                                     bias=1.0)
            # batched tanh (separate ACT table) + v*t multiply
            nc.scalar.activation(dst[:, :, dsl], dst[:, :, dsl], AF.Tanh)
            nc.vector.tensor_tensor(dst[:, :, dsl], vstage[:, :, :width],
                                    dst[:, :, dsl], op=ALU.mult)

        # ---- encode local anchors, kept transposed [feat, m] ----
        hs_sb = spool.tile([128, KD, MPC], F32R)
        enc_layer(hs_sb, W1_sb, b1_sb, sT_sb, 0, MPC)
        se_sb = spool.tile([128, KD, MPC], F32R)
        enc_layer(se_sb, W2_sb, b2_sb, hs_sb, 0, MPC)

        # s2 row: [1, MPC] = sum_d se^2, via ones-vector matmul
        sqse_sb = spool.tile([128, KD, MPC], F32R, tag="hs_sb")
        nc.vector.tensor_tensor(sqse_sb[:, :, :], se_sb[:, :, :],
                                se_sb[:, :, :], op=ALU.mult)
        s2ps = ps.tile([1, 512], F32, tag="tr", bufs=2)
        for k in range(KD):
            nc.tensor.matmul(s2ps[:], ones_col[:], sqse_sb[:, k, :],
                             start=(k == 0), stop=(k == KD - 1))
        s2row_sb = spool.tile([1, MPC], F32)
        nc.vector.tensor_copy(s2row_sb[:], s2ps[:])

        # seA = -2 * se (the gathered distance-matmul stationary operand)
        seA_sb = spool.tile([128, KD, MPC], F32R, tag="sT_sb")
        nc.vector.tensor_scalar_mul(seA_sb[:, :, :], se_sb[:, :, :], -2.0)

        # ---- AllGather of [seA ; s2] ----
        ag_in = dram.tile([AGROWS, MPC], F32)
        ag_out = dram.tile([NCORES * AGROWS, MPC], F32, addr_space="Shared")
        for k in range(KD):
            nc.sync.dma_start(ag_in[128 * k:128 * (k + 1), :].bitcast(F32R),
                              seA_sb[:, k, :])
        nc.sync.dma_start(ag_in[D:D + 1, :], s2row_sb[:])
        nc.gpsimd.collective_compute(
            "AllGather",
            mybir.AluOpType.bypass,
            replica_groups=[list(range(NCORES))],
            ins=[ag_in.opt()],
            outs=[ag_out.opt()],
        )

        # ---- encode x rows (overlaps the AllGather) ----
        hx_sb = xpool.tile([128, KD, RPC], F32R)
        for rc in range(NRC):
            enc_layer(hx_sb, W1_sb, b1_sb, xT_sb, rc, 512)
        xe_sb = xpool.tile([128, KD, RPC], F32R, tag="xT_sb")
        for rc in range(NRC):
            enc_layer(xe_sb, W2_sb, b2_sb, hx_sb, rc, 512)

        # x2 row: [1, RPC] = sum_d xe^2
        sqxe_sb = xpool.tile([128, KD, RPC], F32R, tag="hx_sb")
        nc.vector.tensor_tensor(sqxe_sb[:, :, :], xe_sb[:, :, :],
                                xe_sb[:, :, :], op=ALU.mult)
        x2row_sb = xpool.tile([1, RPC], F32R)
        for rc in range(NRC):
            xps = ps.tile([1, 512], F32, tag="tr", bufs=2)
            for k in range(KD):
                nc.tensor.matmul(xps[:], ones_col[:],
                                 sqxe_sb[:, k, 512 * rc:512 * (rc + 1)],
                                 start=(k == 0), stop=(k == KD - 1))
            nc.vector.tensor_copy(x2row_sb[:, 512 * rc:512 * (rc + 1)],
                                  xps[:])

        # ---- load gathered anchors: seAg[d_part, k, m] for all 4096 m ----
        seAg_sb = gpool.tile([128, KD, M], F32R)
        for k in range(KD):
            for g in range(NCORES):
                nc.sync.dma_start(
                    seAg_sb[:, k, MPC * g:MPC * (g + 1)],
                    ag_out[AGROWS * g + 128 * k:
                           AGROWS * g + 128 * (k + 1), :].bitcast(F32R))
        # s2 columns: s2c[p, t] = s2[128 t + p]
        s2c_sb = gpool.tile([128, NMT], F32)
        for g in range(NCORES):
            row = ag_out[AGROWS * g + D:AGROWS * g + D + 1, :]
            nc.sync.dma_start(
                s2c_sb[:, (MPC // 128) * g:(MPC // 128) * (g + 1)],
                row.rearrange("o (f p) -> (o p) f", p=128))

        # ---- main fused loop: distances + perceptron (transposed).
        # The zT matmul for tile t is emitted after the d2 group of tile
        # t+2 so the in-order PE stream never waits on the sqrt pass. ----
        zpre_sb = zpool.tile([128, 2 * NRC * 2, C], F32, bufs=1)
        for rc in range(NRC):
            zt_ps = psz.tile([C, 512], F32)
            dist_tiles = {}
            for t in range(NMT):
                d2ps = ps.tile([128, 512], F32, tag="mm", bufs=4)
                for k in range(KD):
                    nc.tensor.matmul(d2ps[:],
                                     seAg_sb[:, k, 128 * t:128 * (t + 1)],
                                     xe_sb[:, k, 512 * rc:512 * (rc + 1)],
                                     start=(k == 0), stop=False)
                # += x2[r] (rank-1 augmentation)
                nc.tensor.matmul(d2ps[:], ones512[:, :128],
                                 x2row_sb[:, 512 * rc:512 * (rc + 1)],
                                 start=False, stop=True)
                distT = dpool.tile([128, 512], F32R)
                nc.scalar.activation(distT[:], d2ps[:], AF.Sqrt,
                                     bias=s2c_sb[:, t:t + 1])
                dist_tiles[t] = distT
                if t >= 2:
                    nc.tensor.matmul(zt_ps[:], Wp_sb[:, t - 2, :],
                                     dist_tiles.pop(t - 2)[:],
                                     start=(t - 2 == 0), stop=False,
                                     skip_group_check=True)
            for t in (NMT - 2, NMT - 1):
                nc.tensor.matmul(zt_ps[:], Wp_sb[:, t, :],
                                 dist_tiles.pop(t)[:],
                                 start=False, stop=False,
                                 skip_group_check=True)
            # += bp (rank-1 augmentation), closes the accumulation
            nc.tensor.matmul(zt_ps[:], bp_sb[:], ones512[:],
                             start=False, stop=True, skip_group_check=True)
            # transpose pre-tanh z to [r, C] blocks
            zt_sb = zpool.tile([C, 512], F32, bufs=1)
            nc.vector.tensor_copy(zt_sb[:], zt_ps[:])
            for j in range(4):
                ztr = ps.tile([128, C], F32, tag="tr", bufs=2)
                nc.tensor.matmul(ztr[:], zt_sb[:, 128 * j:128 * (j + 1)],
                                 ident[:], is_transpose=True)
                nc.vector.tensor_copy(zpre_sb[:, 4 * rc + j, :], ztr[:])

        # ---- epilogue: tanh + log-softmax (batched; tanh output is in
        # [-1, 1] so the usual max-subtraction is unnecessary) ----
        NT = 2 * NRC * 2  # 8 tiles of 128 rows
        zth_sb = zpool.tile([128, NT, C], F32, bufs=1)
        nc.scalar.activation(zth_sb[:, :, :], zpre_sb[:, :, :], AF.Tanh)
        e_sb = zpool.tile([128, NT, C], F32, bufs=1, tag="zpre_sb")
        nc.scalar.activation(e_sb[:, :, :], zth_sb[:, :, :], AF.Exp)
        ssum = zpool.tile([128, NT], F32, bufs=1)
        nc.vector.tensor_reduce(ssum[:], e_sb[:, :, :],
                                axis=mybir.AxisListType.X, op=ALU.add)
        lns = zpool.tile([128, NT], F32, bufs=1)
        nc.scalar.activation(lns[:], ssum[:], AF.Ln)
        for jj in range(NT):
            o_sb = zpool.tile([128, C], F32)
            nc.vector.tensor_scalar(o_sb[:], zth_sb[:, jj, :],
                                    lns[:, jj:jj + 1], None,
                                    op0=ALU.subtract)
            nc.sync.dma_start(out[128 * jj:128 * (jj + 1), :], o_sb[:])


_NC_CACHE = None


def _get_nc():
    global _NC_CACHE
    if _NC_CACHE is None:
        _NC_CACHE = build_kernel()
    return _NC_CACHE


def make_in_maps(x, samples, W1, b1, W2, b2, Wp, bp):
    xT = np.ascontiguousarray(np.asarray(x, dtype=np.float32).T)
    sT = np.ascontiguousarray(np.asarray(samples, dtype=np.float32).T)
    W1 = np.ascontiguousarray(np.asarray(W1, dtype=np.float32))
    W2 = np.ascontiguousarray(np.asarray(W2, dtype=np.float32))
    Wp = np.ascontiguousarray(np.asarray(Wp, dtype=np.float32))
    b1c = np.ascontiguousarray(np.asarray(b1, dtype=np.float32).reshape(1, D))
    b2c = np.ascontiguousarray(np.asarray(b2, dtype=np.float32).reshape(1, D))
    bpc = np.ascontiguousarray(np.asarray(bp, dtype=np.float32).reshape(1, C))
    in_maps = []
    for g in range(NCORES):
        in_maps.append({
            "xT": np.ascontiguousarray(xT[:, RPC * g:RPC * (g + 1)]),
            "sT": np.ascontiguousarray(sT[:, MPC * g:MPC * (g + 1)]),
            "W1": W1, "W2": W2, "b1": b1c, "b2": b2c,
            "Wp": Wp, "bp": bpc,
        })
    return in_maps


def run(in_maps, trace=False):
    nc = _get_nc()
    res = bass_utils.run_bass_kernel_spmd(nc, in_maps,
                                          core_ids=list(range(NCORES)),
                                          trace=trace)
    outp = np.concatenate([res.results[g]["out"] for g in range(NCORES)],
                          axis=0).astype(np.float32)
    return outp, res


def kernel(x, samples, W1, b1, W2, b2, Wp, bp):
    in_maps = make_in_maps(x, samples, W1, b1, W2, b2, Wp, bp)
    outp, _ = run(in_maps, trace=False)
    return outp


# revision 8
# speedup vs baseline: 1.7070x; 1.6695x over previous
"""AnchorDML Trainium2 kernel: 8-core SPMD, data-parallel over x rows with
sharded anchor encoding + AllGather of encoded anchors.

Problem (hardcoded):
    N, M, D, C = 8192, 4096, 512, 100
    xe = mish(mish(x @ W1 + b1) @ W2 + b2)          [N, D]
    se = mish(mish(samples @ W1 + b1) @ W2 + b2)    [M, D]
    dist = sqrt(max(|xe|^2 + |se|^2 - 2 xe@se.T, 0))  [N, M]
    out = log_softmax(tanh(dist @ Wp + bp), axis=1)   [N, C]

Sharding: core g handles x rows [1024g, 1024(g+1)) and encodes anchors
[512g, 512(g+1)); encoded (scaled) anchors + |se|^2 are AllGathered.

Precision: encoder and distance GEMM operands are bf16 (fp32 psum
accumulation; errors enter only through operand rounding and add in
quadrature through the distance, ~1e-3 of output scale). The perceptron
GEMM (dist @ Wp) stays float32r because dist ~ 32 is nearly constant, so
Wp rounding would bias whole output columns. |xe|^2 / |se|^2 ride in
fp32 via a DVE add (per-partition s2 + broadcast-row x2).

mish(v) = v * tanh(ln(1 + e^v)) from exp/ln/tanh LUTs (no mish LUT in
this compiler build); tanh is batched per layer to amortize ACT table
loads, and the pre-activation v is staged out of PSUM immediately so
banks recycle fast.
"""
import numpy as np
import ml_dtypes
from concourse import bass, bacc, tile, mybir, bass_utils, masks

N, M, D, C = 8192, 4096, 512, 100
NCORES = 8
RPC = N // NCORES      # 1024 x-rows per core
MPC = M // NCORES      # 512 anchors encoded per core
KD = D // 128          # 4 contraction chunks of 128
NMT = M // 128         # 32 anchor tiles in the distance matmul
NRC = RPC // 512       # 2 row-chunks of 512

F32 = mybir.dt.float32
F32R = mybir.dt.float32r
BF16 = mybir.dt.bfloat16
AF = mybir.ActivationFunctionType
ALU = mybir.AluOpType


def build_kernel():
    nc = bacc.Bacc("TRN2", target_bir_lowering=False, debug=False,
                   num_devices=NCORES)

    xT = nc.dram_tensor("xT", [D, RPC], BF16, kind="ExternalInput")
    sT = nc.dram_tensor("sT", [D, MPC], BF16, kind="ExternalInput")
    W1 = nc.dram_tensor("W1", [D, D], BF16, kind="ExternalInput")
    W2 = nc.dram_tensor("W2", [D, D], BF16, kind="ExternalInput")
    b1 = nc.dram_tensor("b1", [D, 1], F32, kind="ExternalInput")
    b2 = nc.dram_tensor("b2", [D, 1], F32, kind="ExternalInput")
    Wp = nc.dram_tensor("Wp", [M, C], F32, kind="ExternalInput")
    bp = nc.dram_tensor("bp", [1, C], F32, kind="ExternalInput")
    out = nc.dram_tensor("out", [RPC, C], F32, kind="ExternalOutput")

    with tile.TileContext(nc) as tc:
        _body(tc, xT, sT, W1, W2, b1, b2, Wp, bp, out)

    nc.compile()
    return nc


def _body(tc, xT, sT, W1, W2, b1, b2, Wp, bp, out):
    nc = tc.nc
    with (
        tc.tile_pool(name="const", bufs=1) as const,
        tc.tile_pool(name="wpool", bufs=1) as wpool,
        tc.tile_pool(name="spool", bufs=1) as spool,
        tc.tile_pool(name="xpool", bufs=1) as xpool,
        tc.tile_pool(name="gpool", bufs=1) as gpool,
        tc.tile_pool(name="mpool", bufs=2) as mpool,
        tc.tile_pool(name="dpool", bufs=4) as dpool,
        tc.tile_pool(name="zpool", bufs=2) as zpool,
        tc.tile_pool(name="ps", bufs=1, space="PSUM") as ps,
        tc.tile_pool(name="psz", bufs=2, space="PSUM") as psz,
        tc.tile_pool(name="dram", bufs=1, space="DRAM") as dram,
    ):
        # ---- first-needed input loads (encoder layer 1 of the anchors) ----
        sT_sb = spool.tile([128, KD, MPC], BF16)
        for k in range(KD):
            nc.sync.dma_start(sT_sb[:, k, :], sT[128 * k:128 * (k + 1), :])
        W1_sb = wpool.tile([128, KD, D], BF16)
        for k in range(KD):
            nc.sync.dma_start(W1_sb[:, k, :], W1[128 * k:128 * (k + 1), :])
        b1c_sb = wpool.tile([128, KD], F32)
        b2c_sb = wpool.tile([128, KD], F32)
        for k in range(KD):
            nc.sync.dma_start(b1c_sb[:, k:k + 1], b1[128 * k:128 * (k + 1), :])
            nc.sync.dma_start(b2c_sb[:, k:k + 1], b2[128 * k:128 * (k + 1), :])
        W2_sb = wpool.tile([128, KD, D], BF16)
        for k in range(KD):
            nc.sync.dma_start(W2_sb[:, k, :], W2[128 * k:128 * (k + 1), :])
        xT_sb = xpool.tile([128, KD, RPC], BF16)
        for k in range(KD):
            nc.sync.dma_start(xT_sb[:, k, :], xT[128 * k:128 * (k + 1), :])

        # ---- constants ----
        ident = const.tile([C, C], F32)
        masks.make_identity(nc, ident[:])
        ones_f32 = const.tile([128, 1], F32)
        nc.gpsimd.memset(ones_f32[:], 1.0)
        ones_col = const.tile([128, 1], BF16)    # lhsT for row-sum matmuls
        nc.scalar.activation(ones_col[:], ones_f32[:], AF.Copy)
        onesr_f32 = const.tile([1, 512], F32)
        nc.gpsimd.memset(onesr_f32[:], 1.0)
        ones512 = const.tile([1, 512], F32R)     # rhs/lhsT for rank-1 terms
        nc.scalar.activation(ones512[:], onesr_f32[:], AF.Copy)

        # later-needed weights
        bp_sb = wpool.tile([1, C], F32R)
        nc.sync.dma_start(bp_sb[:], bp[:].bitcast(F32R))
        Wp_sb = wpool.tile([128, NMT, C], F32R)
        for t in range(NMT):
            nc.sync.dma_start(Wp_sb[:, t, :],
                              Wp[128 * t:128 * (t + 1), :].bitcast(F32R))

        def enc_layer(dst, Wsb, bcol, src, rc, width):
            """dst[:, :, rc*width:(rc+1)*width] = mish(src.T @ W + b) for all
            KD feature chunks of one layer / row-chunk. The pre-activation v
            is staged to SBUF (with bias) so psum recycles fast; sp=ln(1+e^v)
            lands in dst; tanh + v*t multiply are batched over KD chunks."""
            dsl = slice(width * rc, width * (rc + 1))
            vstage = mpool.tile([128, KD, 512], F32, tag="vstage")
            for f in range(KD):
                vps = ps.tile([128, 512], F32, tag="mm", bufs=4)
                vp = vps[:, :width]
                for k in range(KD):
                    nc.tensor.matmul(vp, Wsb[:, k, 128 * f:128 * (f + 1)],
                                     src[:, k, dsl],
                                     start=(k == 0), stop=(k == KD - 1))
                u = mpool.tile([128, 512], F32, tag="mu", bufs=3)
                nc.scalar.activation(u[:, :width], vp, AF.Exp,
                                     bias=bcol[:, f:f + 1])
                nc.vector.tensor_scalar_add(vstage[:, f, :width], vp,
                                            bcol[:, f:f + 1])
                nc.scalar.activation(dst[:, f, dsl], u[:, :width], AF.Ln,
                                     bias=1.0)
            # batched tanh (separate ACT table) + v*t multiply
            nc.scalar.activation(dst[:, :, dsl], dst[:, :, dsl], AF.Tanh)
            nc.vector.tensor_tensor(dst[:, :, dsl], vstage[:, :, :width],
                                    dst[:, :, dsl], op=ALU.mult)

        # ---- encode local anchors, kept transposed [feat, m] ----
        hs_sb = spool.tile([128, KD, MPC], BF16)
        enc_layer(hs_sb, W1_sb, b1c_sb, sT_sb, 0, MPC)
        se_sb = spool.tile([128, KD, MPC], BF16)
        enc_layer(se_sb, W2_sb, b2c_sb, hs_sb, 0, MPC)

        # s2 row: [1, MPC] = sum_d se^2, via ones-vector matmul
        sqse_sb = spool.tile([128, KD, MPC], BF16, tag="hs_sb")
        nc.vector.tensor_tensor(sqse_sb[:, :, :], se_sb[:, :, :],
                                se_sb[:, :, :], op=ALU.mult)
        s2ps = ps.tile([1, 512], F32, tag="tr", bufs=2)
        for k in range(KD):
            nc.tensor.matmul(s2ps[:], ones_col[:], sqse_sb[:, k, :],
                             start=(k == 0), stop=(k == KD - 1))
        s2row_sb = spool.tile([1, MPC], F32)
        nc.vector.tensor_copy(s2row_sb[:], s2ps[:])

        # seA = -2 * se (the gathered distance-matmul stationary operand)
        seA_sb = spool.tile([128, KD, MPC], BF16, tag="sT_sb")
        nc.vector.tensor_scalar_mul(seA_sb[:, :, :], se_sb[:, :, :], -2.0)

        # ---- AllGather of seA (bf16) and s2 (f32) ----
        ag_in = dram.tile([D, MPC], BF16)
        ag_out = dram.tile([NCORES * D, MPC], BF16, addr_space="Shared")
        for k in range(KD):
            nc.sync.dma_start(ag_in[128 * k:128 * (k + 1), :],
                              seA_sb[:, k, :])
        ag2_in = dram.tile([1, MPC], F32)
        ag2_out = dram.tile([NCORES, MPC], F32, addr_space="Shared")
        nc.sync.dma_start(ag2_in[:], s2row_sb[:])
        nc.gpsimd.collective_compute(
            "AllGather", ALU.bypass,
            replica_groups=[list(range(NCORES))],
            ins=[ag_in.opt()], outs=[ag_out.opt()])
        nc.gpsimd.collective_compute(
            "AllGather", ALU.bypass,
            replica_groups=[list(range(NCORES))],
            ins=[ag2_in.opt()], outs=[ag2_out.opt()])

        # ---- encode x rows (overlaps the AllGather) ----
        hx_sb = xpool.tile([128, KD, RPC], BF16)
        for rc in range(NRC):
            enc_layer(hx_sb, W1_sb, b1c_sb, xT_sb, rc, 512)
        xe_sb = xpool.tile([128, KD, RPC], BF16, tag="xT_sb")
        for rc in range(NRC):
            enc_layer(xe_sb, W2_sb, b2c_sb, hx_sb, rc, 512)

        # x2 broadcast tile: x2b[p, rc, r] = |xe_r|^2 for every partition
        sqxe_sb = xpool.tile([128, KD, RPC], BF16, tag="hx_sb")
        nc.vector.tensor_tensor(sqxe_sb[:, :, :], xe_sb[:, :, :],
                                xe_sb[:, :, :], op=ALU.mult)
        x2row_sb = xpool.tile([1, RPC], F32R)
        x2b_sb = xpool.tile([128, NRC, 512], F32)
        for rc in range(NRC):
            xps = ps.tile([1, 512], F32, tag="tr", bufs=2)
            for k in range(KD):
                nc.tensor.matmul(xps[:], ones_col[:],
                                 sqxe_sb[:, k, 512 * rc:512 * (rc + 1)],
                                 start=(k == 0), stop=(k == KD - 1))
            nc.vector.tensor_copy(x2row_sb[:, 512 * rc:512 * (rc + 1)],
                                  xps[:])
            xbs = ps.tile([128, 512], F32, tag="mm", bufs=4)
            nc.tensor.matmul(xbs[:], ones512[:, :128],
                             x2row_sb[:, 512 * rc:512 * (rc + 1)],
                             start=True, stop=True)
            nc.vector.tensor_copy(x2b_sb[:, rc, :], xbs[:])

        # ---- load gathered anchors: seAg[d_part, k, m] for all 4096 m ----
        seAg_sb = gpool.tile([128, KD, M], BF16)
        for k in range(KD):
            for g in range(NCORES):
                nc.sync.dma_start(
                    seAg_sb[:, k, MPC * g:MPC * (g + 1)],
                    ag_out[D * g + 128 * k:D * g + 128 * (k + 1), :])
        # s2 columns: s2c[p, t] = s2[128 t + p]
        s2c_sb = gpool.tile([128, NMT], F32)
        for g in range(NCORES):
            row = ag2_out[g:g + 1, :]
            nc.sync.dma_start(
                s2c_sb[:, (MPC // 128) * g:(MPC // 128) * (g + 1)],
                row.rearrange("o (f p) -> (o p) f", p=128))

        # ---- main fused loop: distances + perceptron (transposed).
        # The zT matmul for tile t is emitted after the d2 group of tile
        # t+2 so the in-order PE stream never waits on the sqrt pass. ----
        zpre_sb = zpool.tile([128, 2 * NRC * 2, C], F32, bufs=1)
        for rc in range(NRC):
            zt_ps = psz.tile([C, 512], F32)
            dist_tiles = {}
            for t in range(NMT):
                d2ps = ps.tile([128, 512], F32, tag="mm", bufs=4)
                for k in range(KD):
                    nc.tensor.matmul(d2ps[:],
                                     seAg_sb[:, k, 128 * t:128 * (t + 1)],
                                     xe_sb[:, k, 512 * rc:512 * (rc + 1)],
                                     start=(k == 0), stop=(k == KD - 1))
                # d2 += s2[m] (per-partition) + x2[r] (broadcast row), fp32
                nc.vector.scalar_tensor_tensor(
                    d2ps[:], d2ps[:], s2c_sb[:, t:t + 1], x2b_sb[:, rc, :],
                    op0=ALU.add, op1=ALU.add)
                distT = dpool.tile([128, 512], F32R)
                nc.scalar.activation(distT[:], d2ps[:], AF.Sqrt)
                dist_tiles[t] = distT
                if t >= 2:
                    nc.tensor.matmul(zt_ps[:], Wp_sb[:, t - 2, :],
                                     dist_tiles.pop(t - 2)[:],
                                     start=(t - 2 == 0), stop=False,
                                     skip_group_check=True)
            for t in (NMT - 2, NMT - 1):
                nc.tensor.matmul(zt_ps[:], Wp_sb[:, t, :],
                                 dist_tiles.pop(t)[:],
                                 start=False, stop=False,
                                 skip_group_check=True)
            # += bp (rank-1 augmentation), closes the accumulation
            nc.tensor.matmul(zt_ps[:], bp_sb[:], ones512[:],
                             start=False, stop=True, skip_group_check=True)
            # transpose pre-tanh z to [r, C] blocks
            zt_sb = zpool.tile([C, 512], F32, bufs=1)
            nc.vector.tensor_copy(zt_sb[:], zt_ps[:])
            for j in range(4):
                ztr = ps.tile([128, C], F32, tag="tr", bufs=2)
                nc.tensor.matmul(ztr[:], zt_sb[:, 128 * j:128 * (j + 1)],
                                 ident[:], is_transpose=True)
                nc.vector.tensor_copy(zpre_sb[:, 4 * rc + j, :], ztr[:])

        # ---- epilogue: tanh + log-softmax (batched; tanh output is in
        # [-1, 1] so the usual max-subtraction is unnecessary) ----
        NT = 2 * NRC * 2  # 8 tiles of 128 rows
        zth_sb = zpool.tile([128, NT, C], F32, bufs=1)
        nc.scalar.activation(zth_sb[:, :, :], zpre_sb[:, :, :], AF.Tanh)
        e_sb = zpool.tile([128, NT, C], F32, bufs=1, tag="zpre_sb")
        nc.scalar.activation(e_sb[:, :, :], zth_sb[:, :, :], AF.Exp)
        ssum = zpool.tile([128, NT], F32, bufs=1)
        nc.vector.tensor_reduce(ssum[:], e_sb[:, :, :],
                                axis=mybir.AxisListType.X, op=ALU.add)
        lns = zpool.tile([128, NT], F32, bufs=1)
        nc.scalar.activation(lns[:], ssum[:], AF.Ln)
        for jj in range(NT):
            o_sb = zpool.tile([128, C], F32)
            nc.vector.tensor_scalar(o_sb[:], zth_sb[:, jj, :],
                                    lns[:, jj:jj + 1], None,
                                    op0=ALU.subtract)
            nc.sync.dma_start(out[128 * jj:128 * (jj + 1), :], o_sb[:])


_NC_CACHE = None


def _get_nc():
    global _NC_CACHE
    if _NC_CACHE is None:
        _NC_CACHE = build_kernel()
    return _NC_CACHE


def make_in_maps(x, samples, W1, b1, W2, b2, Wp, bp):
    bf = ml_dtypes.bfloat16
    xT = np.ascontiguousarray(np.asarray(x, dtype=np.float32).T.astype(bf))
    sT = np.ascontiguousarray(
        np.asarray(samples, dtype=np.float32).T.astype(bf))
    W1b = np.ascontiguousarray(np.asarray(W1, dtype=np.float32).astype(bf))
    W2b = np.ascontiguousarray(np.asarray(W2, dtype=np.float32).astype(bf))
    Wpc = np.ascontiguousarray(np.asarray(Wp, dtype=np.float32))
    b1c = np.ascontiguousarray(np.asarray(b1, dtype=np.float32).reshape(D, 1))
    b2c = np.ascontiguousarray(np.asarray(b2, dtype=np.float32).reshape(D, 1))
    bpc = np.ascontiguousarray(np.asarray(bp, dtype=np.float32).reshape(1, C))
    in_maps = []
    for g in range(NCORES):
        in_maps.append({
            "xT": np.ascontiguousarray(xT[:, RPC * g:RPC * (g + 1)]),
            "sT": np.ascontiguousarray(sT[:, MPC * g:MPC * (g + 1)]),
            "W1": W1b, "W2": W2b, "b1": b1c, "b2": b2c,
            "Wp": Wpc, "bp": bpc,
        })
    return in_maps


def run(in_maps, trace=False):
    nc = _get_nc()
    res = bass_utils.run_bass_kernel_spmd(nc, in_maps,
                                          core_ids=list(range(NCORES)),
                                          trace=trace)
    outp = np.concatenate([res.results[g]["out"] for g in range(NCORES)],
                          axis=0).astype(np.float32)
    return outp, res


def kernel(x, samples, W1, b1, W2, b2, Wp, bp):
    in_maps = make_in_maps(x, samples, W1, b1, W2, b2, Wp, bp)
    outp, _ = run(in_maps, trace=False)
    return outp
